# revision 13
# baseline (speedup 1.0000x reference)
"""Trainium2 Bass kernel for nn_DAGLayer (gnn_message_passing).

Problem: out buffer holds L leaf columns followed by M computed nodes.
Node i gathers P=8 parent columns (each [N, C]) from the buffer, applies a
per-node dense map y = einsum('ncp,ocp->no', g, W[i]) + b[i], and appends y.

Strategy (8 NeuronCores, one SPMD program):
  - Host schedules nodes into rounds with a LOCALITY rule: a node may depend
    on rounds newer than the last completed AllGather only via parents that
    were computed on the SAME core.  Those "window" parents are gathered from
    a core-local DRAM staging buffer (`own`), so the AllGather is never on
    the round-to-round critical chain - it gets a full AG-window of slack.
  - AllGathers are batched: one AG per ~AG_T slots (covering several rounds).
  - Weights are stored as fp8 e3m4 scaled by 512 (uniform-distributed W fits
    4 mantissa bits; measured DAG rel err ~8.5e-3 vs fp32).  All stored
    activations (leaves and node outputs) are at 1/512 scale so the fp8
    scaling cancels exactly; the host rescales the output by 512.
  - Per slot: 8 gather DMAs (positions [0,a) from `own`, [a,8) from hbuf;
    taps permuted per-core on the host so the split is SPMD-uniform), then
    32 accumulating matmuls (hbuf/old taps FIRST, window/fresh taps LAST so
    only the fresh taps sit on the critical chain), bias+1/512 applied by
    the psum->sbuf activation, y written to `own` (gpsimd; same queue as the
    AGs, so AG inputs are ordered after y writes for free).
  - Engine split: weight DMAs alternate sync/scalar (HWDGE); gathers are
    positions 0-3 sync / 4-7 scalar; activations scalar; leaf init + y
    writes + AllGathers gpsimd; matmuls tensor.

Compute is e3m4[weights] x fp16[activations] with fp32 PSUM accumulation.
The kernel is self-contained; the schedule is derived from the inputs at
run time on the host.
"""

import os

import numpy as np

os.environ.setdefault("NEURON_COMPILE_CACHE_URL", "/root/neuron_cache")

NCORES = 8
AG_T = 5          # AllGather threshold: place an AG top once >= AG_T slots pending
COV_LAG = 1       # AG at top t is readable from round t+COV_LAG
W_PF = 6          # weight DMAs emitted this many rounds ahead of use
W_SCALE = 512.0   # weight scale for e3m4; activations stored at 1/W_SCALE
W_DTYPE = "f8e3"  # "f8e3" or "f16"
A_MAX = 4         # own-positions per slot capped so positions [0,4) are sync

_BUILD_CACHE = {}


# ---------------------------------------------------------------- scheduler
def _compute_schedule(parents, L, M):
    """Rounds of k_r slots per core; AG tops; per-slot own-position count a_s.

    Constraints for node v at (round r, core c):
      - parents with round > cov(r) must be on core c (read from `own`);
        cov(r) = last AG top <= r-1 covers rounds <= top-1.
      - per slot s the tap split [0,a_s)=own / [a_s,8)=hbuf is shared across
        cores: a_s >= #window-parents and the node must have >= a_s parents
        that are leaves or core-local (own-readable), and its remote covered
        parents must fit in the 8-a_s hbuf positions.
    """
    NC = NCORES
    children = [[] for _ in range(L + M)]
    for i in range(M):
        for p in set(parents[i].tolist()):
            children[p].append(L + i)
    height = np.zeros(L + M, np.int64)
    for v in range(L + M - 1, L - 1, -1):
        for ch in children[v]:
            height[v] = max(height[v], height[ch] + 1)
    round_of = np.full(L + M, -1, np.int64)
    core_of = np.full(L + M, -1, np.int64)
    round_of[:L] = -(10**9)
    unsched = set(range(L, L + M))
    core_lock = {}
    rounds = []
    tops = []            # round indices with an AG at their top
    slots_done = 0       # slots of completed rounds
    covered_slots = 0    # slots covered by emitted AGs
    cov_round = [-1]     # cov(r) for current r, updated as tops placed
    r = 0
    guard = 0
    while unsched and guard < 400:
        guard += 1
        # place an AG top at this round?  (rounds must align with r)
        assert len(rounds) == r
        if rounds and (slots_done - covered_slots) >= AG_T:
            tops.append(r)
            covered_slots = slots_done
        # cov(r): rounds covered by AGs at tops <= r - COV_LAG
        c_r = -1
        for t in tops:
            if t <= r - COV_LAG:
                c_r = t - 1
        free, forced = [], {c: [] for c in range(NC)}
        for v in unsched:
            ps = set(parents[v - L].tolist())
            ok = True
            fcores = set()
            for p in ps:
                pr = round_of[p]
                if p < L:
                    continue
                if pr < 0:
                    ok = False
                    break
                if pr > c_r:
                    if pr <= r - 1:
                        fcores.add(core_of[p])
                    else:
                        ok = False
                        break
            if not ok:
                continue
            if v in core_lock:
                fcores.add(core_lock[v])
            if len(fcores) > 1:
                continue
            if len(fcores) == 1:
                forced[list(fcores)[0]].append(v)
            else:
                free.append(v)
        total = len(free) + sum(len(x) for x in forced.values())
        if total == 0:
            # nothing eligible: emit an empty round so indices stay aligned
            rounds.append(np.full((NC, 0), -1, np.int64))
            r += 1
            if r > 300:
                break
            continue
        maxk = 0
        for k in range(1, 65):
            if sum(max(0, k - len(forced[c])) for c in range(NC)) <= len(free):
                maxk = k
        k = min(maxk if maxk else 1, (total + NC - 1) // NC)
        free.sort(key=lambda v: -height[v])
        for c in forced:
            forced[c].sort(key=lambda v: -height[v])
        assign = {c: list(forced[c][:k]) for c in range(NC)}
        fi = 0
        for c in range(NC):
            while len(assign[c]) < k and fi < len(free):
                assign[c].append(free[fi])
                fi += 1

        def stats(v, c):
            f = cap = 0
            for p in parents[v - L]:
                if p < L:
                    cap += 1
                elif round_of[p] > c_r:
                    f += 1
                    cap += 1
                elif core_of[p] == c:
                    cap += 1
            return f, cap

        percore = {c: [(v,) + stats(v, c) for v in assign[c]] for c in range(NC)}
        for c in percore:
            percore[c].sort(key=lambda t: -t[1])
        deferred = []
        stable = False
        while not stable:
            stable = True
            kk = max((len(percore[c]) for c in range(NC)), default=0)
            for m in range(kk):
                col = [(c, percore[c][m]) for c in range(NC) if m < len(percore[c])]
                if not col:
                    continue
                a = max(t[1] for _, t in col)
                bad = [(c, t) for c, t in col if t[2] < a or t[1] > A_MAX]
                if not bad:
                    continue
                stable = False
                for c, t in bad:
                    if t[1] > A_MAX:
                        percore[c] = [x for x in percore[c] if x[0] != t[0]]
                        deferred.append((t[0], c))
                        continue
                    swapped = False
                    for m2 in range(len(percore[c])):
                        if m2 == m:
                            continue
                        t2 = percore[c][m2]
                        col_m = [(cc, x) for cc, x in col if cc != c]
                        col_m2 = [
                            (cc, percore[cc][m2])
                            for cc in range(NC)
                            if m2 < len(percore[cc]) and cc != c
                        ]
                        newa = max([x[1] for _, x in col_m] + [t2[1]])
                        newa2 = max([x[1] for _, x in col_m2] + [t[1]])
                        ok1 = all(x[2] >= newa for _, x in col_m) and t2[2] >= newa
                        ok2 = all(x[2] >= newa2 for _, x in col_m2) and t[2] >= newa2
                        if ok1 and ok2 and newa <= A_MAX and newa2 <= A_MAX:
                            percore[c][m], percore[c][m2] = t2, t
                            swapped = True
                            break
                    if not swapped:
                        percore[c] = [x for x in percore[c] if x[0] != t[0]]
                        deferred.append((t[0], c))
                break
        node_of = np.full((NC, k), -1, np.int64)
        for c in range(NC):
            for m, (v, f, cap) in enumerate(percore[c][:k]):
                node_of[c, m] = v
                round_of[v] = r
                core_of[v] = c
                unsched.discard(v)
        for v, c in deferred:
            core_lock[v] = c
        rounds.append(node_of)
        slots_done += k
        r += 1
    assert not unsched, "scheduler failed to place all nodes"
    return rounds, tops, round_of, core_of


# ---------------------------------------------------------------- bass build
def _build_bass(L, k_list, S, a_list, top_of_round, win_info):
    """k_list[r] = slots of round r; a_list[s] = own positions of slot s;
    top_of_round[r] = (off0, off1) if an AG is emitted at the top of round r
    (covering per-core slots [off0, off1)), else None.
    win_info unused here (host-side layout only)."""
    import concourse.bacc as bacc
    import concourse.bass as bass
    import concourse.mybir as mybir
    import concourse.tile as tile

    f16 = mybir.dt.float16
    f32 = mybir.dt.float32
    i32 = mybir.dt.int32
    f8 = mybir.dt.float8e3 if W_DTYPE == "f8e3" else mybir.dt.float16

    nc = bacc.Bacc(num_devices=NCORES, num_swdge_queues=4)
    OWN_ROWS = (L + S) * 128
    HB_ROWS = (L + 8 * S) * 128

    wbuf = nc.dram_tensor("wbuf", [S, 128, 16, 2, 128], f8, kind="ExternalInput")
    xt = nc.dram_tensor("xt", [L * 128, 64], f16, kind="ExternalInput")
    bbuf = nc.dram_tensor("bbuf", [128, 2 * S], f32, kind="ExternalInput")
    gidx = nc.dram_tensor("gidx", [1, 8 * S], i32, kind="ExternalInput")
    yout = nc.dram_tensor("yout", [S * 128, 64], f16, kind="ExternalOutput")
    own = nc.dram_tensor("own", [OWN_ROWS, 64], f16)
    hbuf = nc.dram_tensor("hbuf", [HB_ROWS, 64], f16, addr_space="Shared")
    rg = [list(range(NCORES))]

    # weight pool must hold W_PF+1 rounds of in-flight slots
    w_bufs = max(
        sum(k_list[r : r + W_PF + 1]) for r in range(len(k_list))
    ) + 1
    with tile.TileContext(nc) as tc:
        with (
            tc.tile_pool(name="const", bufs=1) as constp,
            tc.tile_pool(name="w", bufs=w_bufs) as wp,
            tc.tile_pool(name="g", bufs=10) as gp,
            tc.tile_pool(name="y", bufs=8) as yp,
            tc.tile_pool(name="py", bufs=4, space="PSUM") as pyp,
        ):
            b_sb = constp.tile([128, 2 * S], f32)
            nc.sync.dma_start(b_sb[:], bbuf[:])
            gidx_sb = constp.tile([1, 8 * S], i32)
            nc.sync.dma_start(gidx_sb[:], gidx[:])
            # leaves into the local own buffer (scalar) + shared hbuf (gpsimd)
            nc.scalar.dma_start(own[0 : L * 128, :], xt[:])
            nc.gpsimd.dma_start(hbuf[0 : L * 128, :], xt[:])

            def emit_gathers(s, g_all, positions, region):
                """region: 'own' or 'hbuf'; engine: pos<4 -> sync else scalar."""
                for eng, lo, hi in ((nc.sync, 0, 4), (nc.scalar, 4, 8)):
                    use = [p for p in positions if lo <= p < hi]
                    if not use:
                        continue
                    p0, p1 = use[0], use[-1]
                    cols = gidx_sb[0:1, 8 * s + p0 : 8 * s + p1 + 1]
                    maxv = (OWN_ROWS if region == "own" else HB_ROWS) - 128
                    _, vals = nc.values_load_multi_w_load_instructions(
                        cols,
                        engines=[eng.engine],
                        min_val=0,
                        max_val=maxv,
                        skip_runtime_bounds_check=True,
                    )
                    src = own if region == "own" else hbuf
                    for p in use:
                        eng.dma_start(
                            g_all[:, p, :], src[bass.ds(vals[p - p0], 128), :]
                        )

            R = len(k_list)
            off_of = [0]
            for k_r in k_list:
                off_of.append(off_of[-1] + k_r)
            w_tiles = {}
            g_tiles = {}

            def emit_weights(r):
                if r >= R:
                    return
                for s in range(off_of[r], off_of[r + 1]):
                    w_t = wp.tile([128, 16, 2, 128], f8, tag="w")
                    (nc.sync if (s % 2 == 0) else nc.scalar).dma_start(
                        w_t[:], wbuf[s]
                    )
                    w_tiles[s] = w_t

            def emit_hbuf_gathers(r):
                if r >= R:
                    return
                for s in range(off_of[r], off_of[r + 1]):
                    g_all = gp.tile([128, 8, 64], f16, tag="g")
                    emit_gathers(s, g_all, list(range(a_list[s], 8)), "hbuf")
                    g_tiles[s] = g_all

            # prologue: weights for the first W_PF rounds
            for r0 in range(min(W_PF, R)):
                emit_weights(r0)

            for r, k_r in enumerate(k_list):
                # 1) weights W_PF rounds ahead, then this round's hbuf
                #    gathers - BEFORE this round's AG so they never wait on it
                emit_weights(r + W_PF)
                emit_hbuf_gathers(r)
                # 3) AllGather at this round's top (covers rounds <= r-1;
                #    first readers are round r+1's gathers, emitted next round)
                if top_of_round[r] is not None:
                    o0, o1 = top_of_round[r]
                    nc.gpsimd.collective_compute(
                        "AllGather",
                        mybir.AluOpType.bypass,
                        replica_groups=rg,
                        ins=[own[(L + o0) * 128 : (L + o1) * 128, :]],
                        outs=[
                            hbuf[(L + 8 * o0) * 128 : (L + 8 * o1) * 128, :]
                        ],
                    )
                # 4) own gathers (positions [0, a_s)) + 5) compute
                for s in range(off_of[r], off_of[r + 1]):
                    w_t, g_all = w_tiles.pop(s), g_tiles.pop(s)
                    a_s = a_list[s]
                    emit_gathers(s, g_all, list(range(0, a_s)), "own")
                    # matmul order: hbuf (old) taps first, own (fresh) last
                    pos_order = list(range(a_s, 8)) + list(range(0, a_s))
                    th_order = [(p, h) for p in pos_order for h in range(2)]
                    pys = [
                        pyp.tile([128, 32], f32, tag="py", name=f"py{oh}")
                        for oh in range(2)
                    ]
                    for i, (p, h) in enumerate(th_order):
                        for oh in range(2):
                            nc.tensor.matmul(
                                pys[oh][:],
                                w_t[:, 2 * p + h, oh, :],
                                g_all[:, p, h * 32 : (h + 1) * 32],
                                start=(i == 0),
                                stop=(i == 15),
                            )
                    y16 = yp.tile([128, 2, 32], f16, tag="y16")
                    for oh in range(2):
                        bias = b_sb[:, 2 * s + oh : 2 * s + oh + 1]
                        nc.scalar.activation(
                            y16[:, oh, :],
                            pys[oh][:],
                            mybir.ActivationFunctionType.Identity,
                            bias=bias,
                            scale=float(1.0 / W_SCALE),
                        )
                    nc.gpsimd.dma_start(
                        own[(L + s) * 128 : (L + s + 1) * 128, :], y16[:]
                    )

            nc.sync.dma_start(yout[:], own[L * 128 : (L + S) * 128, :])
    nc.compile()
    return nc


# ---------------------------------------------------------------- host glue
def kernel(x, W, b, parents):
    import ml_dtypes
    from concourse.bass_utils import run_bass_kernel_spmd

    x = np.ascontiguousarray(np.asarray(x), dtype=np.float32)
    W = np.ascontiguousarray(np.asarray(W), dtype=np.float32)
    b = np.ascontiguousarray(np.asarray(b), dtype=np.float32)
    parents = np.asarray(parents).astype(np.int64)

    N, C, L = x.shape
    M, O, C2, P = W.shape
    assert (N, C, O, C2, P) == (32, 256, 256, 256, 8), "kernel hardcodes these dims"

    rounds, tops, round_of, core_of = _compute_schedule(parents, L, M)
    k_list = [nd.shape[1] for nd in rounds]
    R = len(rounds)
    S = sum(k_list)
    off_of_round = np.concatenate([[0], np.cumsum(k_list)]).astype(np.int64)

    # per-core slot index of each node; global hbuf slot via AG windows
    slot_of = np.full(L + M, -1, np.int64)  # per-core slot s
    node_of_coreslot = np.full((NCORES, S), -1, np.int64)
    round_of_slot = np.zeros(S, np.int64)
    for r, nd in enumerate(rounds):
        for m in range(nd.shape[1]):
            s = off_of_round[r] + m
            round_of_slot[s] = r
            for q in range(NCORES):
                v = nd[q, m]
                if v >= 0:
                    slot_of[v] = s
                    node_of_coreslot[q, s] = v

    # AG windows: top at round t covers per-core slots [off0, off1)
    # hbuf layout (after leaves): for window j, rank-major:
    #   row of (core q, slot s) = L + 8*off0_j + q*win_j + (s - off0_j)
    top_of_round = [None] * R
    win_of_slot = np.full(S, -1, np.int64)
    wins = []
    prev = 0
    for t in tops:
        o0, o1 = prev, int(off_of_round[t])
        if o1 > o0:
            top_of_round[t] = (o0, o1)
            wins.append((o0, o1))
            win_of_slot[o0:o1] = len(wins) - 1
            prev = o1

    def hbuf_row(q, s):
        j = win_of_slot[s]
        assert j >= 0
        o0, o1 = wins[j]
        return (L + 8 * o0 + q * (o1 - o0) + (s - o0)) * 128

    # cov(r) for tap classification (must match scheduler's view)
    def cov(r):
        c = -1
        for t in tops:
            if t <= r - COV_LAG:
                c = t - 1
        return c

    # per (core, slot): tap permutation + a_s (shared across cores)
    a_list = np.zeros(S, np.int64)
    perm = np.zeros((NCORES, S, P), np.int64)  # position -> original tap
    gidx_vals = np.zeros((NCORES, S, P), np.int64)
    for s in range(S):
        r = round_of_slot[s]
        c_r = cov(r)
        # a_s = max over cores of #window parents
        amax = 0
        for q in range(NCORES):
            v = node_of_coreslot[q, s]
            if v < 0:
                continue
            nf = sum(
                1 for p in parents[v - L] if p >= L and round_of[p] > c_r
            )
            amax = max(amax, nf)
        a_list[s] = amax
        for q in range(NCORES):
            v = node_of_coreslot[q, s]
            if v < 0:
                perm[q, s] = np.arange(P)
                gidx_vals[q, s] = 0
                continue
            ps = parents[v - L]
            window, local_old, leaf, remote = [], [], [], []
            for t_i, p in enumerate(ps):
                if p < L:
                    leaf.append(t_i)
                elif round_of[p] > c_r:
                    assert core_of[p] == q and round_of[p] < r
                    window.append(t_i)
                elif core_of[p] == q:
                    local_old.append(t_i)
                else:
                    remote.append(t_i)
            own_side = window + local_old + leaf  # priority for own positions
            need = amax
            own_taps = own_side[:need]
            assert len(own_taps) == need, (
                f"slot {s} core {q}: cannot fill {need} own positions"
            )
            rest = [t_i for t_i in range(P) if t_i not in own_taps]
            # rest must be hbuf-eligible: leaf or covered computed
            for t_i in rest:
                p = ps[t_i]
                assert p < L or round_of[p] <= c_r, "window tap in hbuf position"
            order = own_taps + rest
            perm[q, s] = order
            for pos, t_i in enumerate(order):
                p = ps[t_i]
                if pos < amax:  # own region
                    if p < L:
                        gidx_vals[q, s, pos] = p * 128
                    else:
                        gidx_vals[q, s, pos] = (L + slot_of[p]) * 128
                else:  # hbuf region
                    if p < L:
                        gidx_vals[q, s, pos] = p * 128
                    else:
                        gidx_vals[q, s, pos] = hbuf_row(core_of[p], slot_of[p])
    assert a_list.max() <= A_MAX

    # ---- weight relayout: [M, o, c, p] -> [128, 16(ktile=2*pos+h), 2(oh), 128]
    # with tap permutation applied per (core, slot).
    W4 = W.transpose(0, 3, 2, 1).reshape(M, 8, 2, 128, 2, 128)
    # W4[m, tap, h(c//128), c%128, oh, o%128]
    if W_DTYPE == "f8e3":
        W4q = np.clip(W4 * W_SCALE, -15.5, 15.5).astype(ml_dtypes.float8_e3m4)
        wdt = ml_dtypes.float8_e3m4
    else:
        W4q = (W4 * W_SCALE).astype(np.float16)
        wdt = np.float16
    xt_host = np.ascontiguousarray(
        (x.transpose(2, 1, 0) / W_SCALE)
        .reshape(L, 2, 128, 32)
        .transpose(0, 2, 1, 3)
        .reshape(L * 128, 64)
        .astype(np.float16)
    )

    in_maps = []
    for q in range(NCORES):
        nodes_q = node_of_coreslot[q]
        Wq = np.zeros((S, 128, 16, 2, 128), wdt)
        bq = np.zeros((S, 2, 128), np.float32)
        for s in range(S):
            v = nodes_q[s]
            if v < 0:
                continue
            # build [128, 16, 2, 128]: ktile = 2*pos + h -> W4q[v, perm[pos], h]
            wv = W4q[v - L]  # [8, 2, 128, 2, 128]
            wp_ = wv[perm[q, s]]  # [8(pos), 2(h), 128(part), 2(oh), 128(o)]
            Wq[s] = wp_.reshape(16, 128, 2, 128).transpose(1, 0, 2, 3)
            bq[s] = (b[v - L] / W_SCALE).reshape(2, 128)
        bq2 = np.ascontiguousarray(bq.transpose(2, 0, 1).reshape(128, 2 * S))
        gq = np.ascontiguousarray(
            gidx_vals[q].reshape(1, 8 * S).astype(np.int32)
        )
        in_maps.append({"wbuf": Wq, "xt": xt_host, "bbuf": bq2, "gidx": gq})

    key = (
        L,
        tuple(k_list),
        tuple(a_list.tolist()),
        tuple(tops),
        W_DTYPE,
    )
    if key not in _BUILD_CACHE:
        import time as _time

        _t0 = _time.time()
        _BUILD_CACHE[key] = _build_bass(
            L, k_list, S, a_list.tolist(), top_of_round, wins
        )
        print(f"[kernel] bass build took {_time.time() - _t0:.1f}s", flush=True)
    nc = _BUILD_CACHE[key]

    global LAST_RUN
    LAST_RUN = (nc, in_maps)

    results = run_bass_kernel_spmd(nc, in_maps, core_ids=list(range(NCORES))).results

    out = np.zeros((N, C, L + M), np.float32)
    out[:, :, :L] = x
    for q in range(NCORES):
        yq = (
            np.asarray(results[q]["yout"])
            .astype(np.float32)
            .reshape(S, 128, 2, 32)
            .transpose(0, 3, 2, 1)
            .reshape(S, 32, 256)
        ) * W_SCALE
        for s in range(S):
            v = node_of_coreslot[q, s]
            if v >= 0:
                out[:, :, v] = yq[s]  # v is already L-based
    return out


# revision 17
# speedup vs baseline: 1.0079x; 1.0079x over previous
"""Trainium2 Bass kernel for nn_DAGLayer (gnn_message_passing).

Problem: out buffer holds L leaf columns followed by M computed nodes.
Node i gathers P=8 parent columns (each [N, C]) from the buffer, applies a
per-node dense map y = einsum('ncp,ocp->no', g, W[i]) + b[i], and appends y.

Strategy (8 NeuronCores, one SPMD program):
  - Host schedules nodes into rounds with a LOCALITY rule: a node may depend
    on rounds newer than the last completed AllGather only via parents that
    were computed on the SAME core.  Those "window" parents are gathered from
    a core-local DRAM staging buffer (`own`), so the AllGather is never on
    the round-to-round critical chain - it gets a full AG-window of slack.
  - AllGathers are batched: one AG per ~AG_T slots (covering several rounds).
  - Weights are stored as fp8 e3m4 scaled by 512 (uniform-distributed W fits
    4 mantissa bits; measured DAG rel err ~8.5e-3 vs fp32).  All stored
    activations (leaves and node outputs) are at 1/512 scale so the fp8
    scaling cancels exactly; the host rescales the output by 512.
  - Per slot: 8 gather DMAs (positions [0,a) from `own`, [a,8) from hbuf;
    taps permuted per-core on the host so the split is SPMD-uniform), then
    32 accumulating matmuls (hbuf/old taps FIRST, window/fresh taps LAST so
    only the fresh taps sit on the critical chain), bias+1/512 applied by
    the psum->sbuf activation, y written to `own` (gpsimd; same queue as the
    AGs, so AG inputs are ordered after y writes for free).
  - Engine split: weight DMAs alternate sync/scalar (HWDGE); gathers are
    positions 0-3 sync / 4-7 scalar; activations scalar; leaf init + y
    writes + AllGathers gpsimd; matmuls tensor.

Compute is e3m4[weights] x fp16[activations] with fp32 PSUM accumulation.
The kernel is self-contained; the schedule is derived from the inputs at
run time on the host.
"""

import os

import numpy as np

os.environ.setdefault("NEURON_COMPILE_CACHE_URL", "/root/neuron_cache")

NCORES = 8
AG_T = 6          # AllGather threshold: place an AG top once >= AG_T slots pending
COV_LAG = 1       # AG at top t is readable from round t+COV_LAG
W_PF = 6          # weight DMAs emitted this many rounds ahead of use
W_SCALE = 512.0   # weight scale for e3m4; activations stored at 1/W_SCALE
W_DTYPE = "f8e3"  # "f8e3" or "f16"
A_MAX = 4         # own-positions per slot capped so positions [0,4) are sync

_BUILD_CACHE = {}


# ---------------------------------------------------------------- scheduler
def _compute_schedule(parents, L, M):
    """Rounds of k_r slots per core; AG tops; per-slot own-position count a_s.

    Constraints for node v at (round r, core c):
      - parents with round > cov(r) must be on core c (read from `own`);
        cov(r) = last AG top <= r-1 covers rounds <= top-1.
      - per slot s the tap split [0,a_s)=own / [a_s,8)=hbuf is shared across
        cores: a_s >= #window-parents and the node must have >= a_s parents
        that are leaves or core-local (own-readable), and its remote covered
        parents must fit in the 8-a_s hbuf positions.
    """
    NC = NCORES
    children = [[] for _ in range(L + M)]
    for i in range(M):
        for p in set(parents[i].tolist()):
            children[p].append(L + i)
    height = np.zeros(L + M, np.int64)
    for v in range(L + M - 1, L - 1, -1):
        for ch in children[v]:
            height[v] = max(height[v], height[ch] + 1)
    round_of = np.full(L + M, -1, np.int64)
    core_of = np.full(L + M, -1, np.int64)
    round_of[:L] = -(10**9)
    unsched = set(range(L, L + M))
    core_lock = {}
    rounds = []
    tops = []            # round indices with an AG at their top
    slots_done = 0       # slots of completed rounds
    covered_slots = 0    # slots covered by emitted AGs
    cov_round = [-1]     # cov(r) for current r, updated as tops placed
    r = 0
    guard = 0
    while unsched and guard < 400:
        guard += 1
        # place an AG top at this round?  (rounds must align with r)
        assert len(rounds) == r
        if rounds and (slots_done - covered_slots) >= AG_T:
            tops.append(r)
            covered_slots = slots_done
        # cov(r): rounds covered by AGs at tops <= r - COV_LAG
        c_r = -1
        for t in tops:
            if t <= r - COV_LAG:
                c_r = t - 1
        free, forced = [], {c: [] for c in range(NC)}
        for v in unsched:
            ps = set(parents[v - L].tolist())
            ok = True
            fcores = set()
            for p in ps:
                pr = round_of[p]
                if p < L:
                    continue
                if pr < 0:
                    ok = False
                    break
                if pr > c_r:
                    if pr <= r - 1:
                        fcores.add(core_of[p])
                    else:
                        ok = False
                        break
            if not ok:
                continue
            if v in core_lock:
                fcores.add(core_lock[v])
            if len(fcores) > 1:
                continue
            if len(fcores) == 1:
                forced[list(fcores)[0]].append(v)
            else:
                free.append(v)
        total = len(free) + sum(len(x) for x in forced.values())
        if total == 0:
            # nothing eligible: emit an empty round so indices stay aligned
            rounds.append(np.full((NC, 0), -1, np.int64))
            r += 1
            if r > 300:
                break
            continue
        maxk = 0
        for k in range(1, 65):
            if sum(max(0, k - len(forced[c])) for c in range(NC)) <= len(free):
                maxk = k
        k = min(maxk if maxk else 1, (total + NC - 1) // NC)
        free.sort(key=lambda v: -height[v])
        for c in forced:
            forced[c].sort(key=lambda v: -height[v])
        assign = {c: list(forced[c][:k]) for c in range(NC)}
        fi = 0
        for c in range(NC):
            while len(assign[c]) < k and fi < len(free):
                assign[c].append(free[fi])
                fi += 1

        def stats(v, c):
            f = cap = 0
            for p in parents[v - L]:
                if p < L:
                    cap += 1
                elif round_of[p] > c_r:
                    f += 1
                    cap += 1
                elif core_of[p] == c:
                    cap += 1
            return f, cap

        percore = {c: [(v,) + stats(v, c) for v in assign[c]] for c in range(NC)}
        for c in percore:
            percore[c].sort(key=lambda t: -t[1])
        deferred = []
        stable = False
        while not stable:
            stable = True
            kk = max((len(percore[c]) for c in range(NC)), default=0)
            for m in range(kk):
                col = [(c, percore[c][m]) for c in range(NC) if m < len(percore[c])]
                if not col:
                    continue
                a = max(t[1] for _, t in col)
                bad = [(c, t) for c, t in col if t[2] < a or t[1] > A_MAX]
                if not bad:
                    continue
                stable = False
                for c, t in bad:
                    if t[1] > A_MAX:
                        percore[c] = [x for x in percore[c] if x[0] != t[0]]
                        deferred.append((t[0], c))
                        continue
                    swapped = False
                    for m2 in range(len(percore[c])):
                        if m2 == m:
                            continue
                        t2 = percore[c][m2]
                        col_m = [(cc, x) for cc, x in col if cc != c]
                        col_m2 = [
                            (cc, percore[cc][m2])
                            for cc in range(NC)
                            if m2 < len(percore[cc]) and cc != c
                        ]
                        newa = max([x[1] for _, x in col_m] + [t2[1]])
                        newa2 = max([x[1] for _, x in col_m2] + [t[1]])
                        ok1 = all(x[2] >= newa for _, x in col_m) and t2[2] >= newa
                        ok2 = all(x[2] >= newa2 for _, x in col_m2) and t[2] >= newa2
                        if ok1 and ok2 and newa <= A_MAX and newa2 <= A_MAX:
                            percore[c][m], percore[c][m2] = t2, t
                            swapped = True
                            break
                    if not swapped:
                        percore[c] = [x for x in percore[c] if x[0] != t[0]]
                        deferred.append((t[0], c))
                break
        node_of = np.full((NC, k), -1, np.int64)
        for c in range(NC):
            for m, (v, f, cap) in enumerate(percore[c][:k]):
                node_of[c, m] = v
                round_of[v] = r
                core_of[v] = c
                unsched.discard(v)
        for v, c in deferred:
            core_lock[v] = c
        rounds.append(node_of)
        slots_done += k
        r += 1
    assert not unsched, "scheduler failed to place all nodes"
    return rounds, tops, round_of, core_of


# ---------------------------------------------------------------- bass build
def _build_bass(L, k_list, S, a_list, top_of_round, win_info):
    """k_list[r] = slots of round r; a_list[s] = own positions of slot s;
    top_of_round[r] = (off0, off1) if an AG is emitted at the top of round r
    (covering per-core slots [off0, off1)), else None.
    win_info unused here (host-side layout only)."""
    import concourse.bacc as bacc
    import concourse.bass as bass
    import concourse.mybir as mybir
    import concourse.tile as tile

    f16 = mybir.dt.float16
    f32 = mybir.dt.float32
    i32 = mybir.dt.int32
    f8 = mybir.dt.float8e3 if W_DTYPE == "f8e3" else mybir.dt.float16

    nc = bacc.Bacc(num_devices=NCORES, num_swdge_queues=4)
    OWN_ROWS = (L + S) * 128
    HB_ROWS = (L + 8 * S) * 128

    wbuf = nc.dram_tensor("wbuf", [S, 128, 16, 2, 128], f8, kind="ExternalInput")
    xt = nc.dram_tensor("xt", [L * 128, 64], f16, kind="ExternalInput")
    bbuf = nc.dram_tensor("bbuf", [128, 2 * S], f32, kind="ExternalInput")
    gidx = nc.dram_tensor("gidx", [1, 8 * S], i32, kind="ExternalInput")
    yout = nc.dram_tensor("yout", [S * 128, 64], f16, kind="ExternalOutput")
    own = nc.dram_tensor("own", [OWN_ROWS, 64], f16)
    hbuf = nc.dram_tensor("hbuf", [HB_ROWS, 64], f16, addr_space="Shared")
    rg = [list(range(NCORES))]

    # weight pool must hold W_PF+1 rounds of in-flight slots
    w_bufs = max(
        sum(k_list[r : r + W_PF + 1]) for r in range(len(k_list))
    ) + 1
    with tile.TileContext(nc) as tc:
        with (
            tc.tile_pool(name="const", bufs=1) as constp,
            tc.tile_pool(name="w", bufs=w_bufs) as wp,
            tc.tile_pool(name="g", bufs=10) as gp,
            tc.tile_pool(name="y", bufs=8) as yp,
            tc.tile_pool(name="py", bufs=4, space="PSUM") as pyp,
        ):
            b_sb = constp.tile([128, 2 * S], f32)
            nc.sync.dma_start(b_sb[:], bbuf[:])
            gidx_sb = constp.tile([1, 8 * S], i32)
            nc.sync.dma_start(gidx_sb[:], gidx[:])
            # leaves into the local own buffer (scalar) + shared hbuf (gpsimd)
            nc.scalar.dma_start(own[0 : L * 128, :], xt[:])
            nc.gpsimd.dma_start(hbuf[0 : L * 128, :], xt[:])

            def emit_gathers(s, g_all, positions, region):
                """region: 'own' or 'hbuf'; engine: pos<4 -> sync else scalar."""
                for eng, lo, hi in ((nc.sync, 0, 4), (nc.scalar, 4, 8)):
                    use = [p for p in positions if lo <= p < hi]
                    if not use:
                        continue
                    p0, p1 = use[0], use[-1]
                    cols = gidx_sb[0:1, 8 * s + p0 : 8 * s + p1 + 1]
                    maxv = (OWN_ROWS if region == "own" else HB_ROWS) - 128
                    _, vals = nc.values_load_multi_w_load_instructions(
                        cols,
                        engines=[eng.engine],
                        min_val=0,
                        max_val=maxv,
                        skip_runtime_bounds_check=True,
                    )
                    src = own if region == "own" else hbuf
                    for p in use:
                        eng.dma_start(
                            g_all[:, p, :], src[bass.ds(vals[p - p0], 128), :]
                        )

            R = len(k_list)
            off_of = [0]
            for k_r in k_list:
                off_of.append(off_of[-1] + k_r)
            w_tiles = {}
            g_tiles = {}

            def emit_weights(r):
                if r >= R:
                    return
                for s in range(off_of[r], off_of[r + 1]):
                    w_t = wp.tile([128, 16, 2, 128], f8, tag="w")
                    nc.gpsimd.dma_start(w_t[:], wbuf[s])
                    w_tiles[s] = w_t

            def emit_hbuf_gathers(r):
                if r >= R:
                    return
                for s in range(off_of[r], off_of[r + 1]):
                    g_all = gp.tile([128, 8, 64], f16, tag="g")
                    emit_gathers(s, g_all, list(range(a_list[s], 8)), "hbuf")
                    g_tiles[s] = g_all

            # prologue: weights for the first W_PF rounds
            for r0 in range(min(W_PF, R)):
                emit_weights(r0)

            for r, k_r in enumerate(k_list):
                # 1) weights W_PF rounds ahead, then this round's hbuf
                #    gathers - BEFORE this round's AG so they never wait on it
                emit_weights(r + W_PF)
                emit_hbuf_gathers(r)
                # 3) AllGather at this round's top (covers rounds <= r-1;
                #    first readers are round r+1's gathers, emitted next round)
                if top_of_round[r] is not None:
                    o0, o1 = top_of_round[r]
                    nc.gpsimd.collective_compute(
                        "AllGather",
                        mybir.AluOpType.bypass,
                        replica_groups=rg,
                        ins=[own[(L + o0) * 128 : (L + o1) * 128, :]],
                        outs=[
                            hbuf[(L + 8 * o0) * 128 : (L + 8 * o1) * 128, :]
                        ],
                    )
                # 4) own gathers (positions [0, a_s)) + 5) compute
                for s in range(off_of[r], off_of[r + 1]):
                    w_t, g_all = w_tiles.pop(s), g_tiles.pop(s)
                    a_s = a_list[s]
                    emit_gathers(s, g_all, list(range(0, a_s)), "own")
                    # matmul order: hbuf (old) taps first, own (fresh) last
                    pos_order = list(range(a_s, 8)) + list(range(0, a_s))
                    th_order = [(p, h) for p in pos_order for h in range(2)]
                    pys = [
                        pyp.tile([128, 32], f32, tag="py", name=f"py{oh}")
                        for oh in range(2)
                    ]
                    for i, (p, h) in enumerate(th_order):
                        for oh in range(2):
                            nc.tensor.matmul(
                                pys[oh][:],
                                w_t[:, 2 * p + h, oh, :],
                                g_all[:, p, h * 32 : (h + 1) * 32],
                                start=(i == 0),
                                stop=(i == 15),
                            )
                    y16 = yp.tile([128, 2, 32], f16, tag="y16")
                    for oh in range(2):
                        bias = b_sb[:, 2 * s + oh : 2 * s + oh + 1]
                        nc.scalar.activation(
                            y16[:, oh, :],
                            pys[oh][:],
                            mybir.ActivationFunctionType.Identity,
                            bias=bias,
                            scale=float(1.0 / W_SCALE),
                        )
                    nc.gpsimd.dma_start(
                        own[(L + s) * 128 : (L + s + 1) * 128, :], y16[:]
                    )

            nc.sync.dma_start(yout[:], own[L * 128 : (L + S) * 128, :])
    nc.compile()
    return nc


# ---------------------------------------------------------------- host glue
def kernel(x, W, b, parents):
    import ml_dtypes
    from concourse.bass_utils import run_bass_kernel_spmd

    x = np.ascontiguousarray(np.asarray(x), dtype=np.float32)
    W = np.ascontiguousarray(np.asarray(W), dtype=np.float32)
    b = np.ascontiguousarray(np.asarray(b), dtype=np.float32)
    parents = np.asarray(parents).astype(np.int64)

    N, C, L = x.shape
    M, O, C2, P = W.shape
    assert (N, C, O, C2, P) == (32, 256, 256, 256, 8), "kernel hardcodes these dims"

    rounds, tops, round_of, core_of = _compute_schedule(parents, L, M)
    k_list = [nd.shape[1] for nd in rounds]
    R = len(rounds)
    S = sum(k_list)
    off_of_round = np.concatenate([[0], np.cumsum(k_list)]).astype(np.int64)

    # per-core slot index of each node; global hbuf slot via AG windows
    slot_of = np.full(L + M, -1, np.int64)  # per-core slot s
    node_of_coreslot = np.full((NCORES, S), -1, np.int64)
    round_of_slot = np.zeros(S, np.int64)
    for r, nd in enumerate(rounds):
        for m in range(nd.shape[1]):
            s = off_of_round[r] + m
            round_of_slot[s] = r
            for q in range(NCORES):
                v = nd[q, m]
                if v >= 0:
                    slot_of[v] = s
                    node_of_coreslot[q, s] = v

    # AG windows: top at round t covers per-core slots [off0, off1)
    # hbuf layout (after leaves): for window j, rank-major:
    #   row of (core q, slot s) = L + 8*off0_j + q*win_j + (s - off0_j)
    top_of_round = [None] * R
    win_of_slot = np.full(S, -1, np.int64)
    wins = []
    prev = 0
    for t in tops:
        o0, o1 = prev, int(off_of_round[t])
        if o1 > o0:
            top_of_round[t] = (o0, o1)
            wins.append((o0, o1))
            win_of_slot[o0:o1] = len(wins) - 1
            prev = o1

    def hbuf_row(q, s):
        j = win_of_slot[s]
        assert j >= 0
        o0, o1 = wins[j]
        return (L + 8 * o0 + q * (o1 - o0) + (s - o0)) * 128

    # cov(r) for tap classification (must match scheduler's view)
    def cov(r):
        c = -1
        for t in tops:
            if t <= r - COV_LAG:
                c = t - 1
        return c

    # per (core, slot): tap permutation + a_s (shared across cores)
    a_list = np.zeros(S, np.int64)
    perm = np.zeros((NCORES, S, P), np.int64)  # position -> original tap
    gidx_vals = np.zeros((NCORES, S, P), np.int64)
    for s in range(S):
        r = round_of_slot[s]
        c_r = cov(r)
        # a_s = max over cores of #window parents
        amax = 0
        for q in range(NCORES):
            v = node_of_coreslot[q, s]
            if v < 0:
                continue
            nf = sum(
                1 for p in parents[v - L] if p >= L and round_of[p] > c_r
            )
            amax = max(amax, nf)
        a_list[s] = amax
        for q in range(NCORES):
            v = node_of_coreslot[q, s]
            if v < 0:
                perm[q, s] = np.arange(P)
                gidx_vals[q, s] = 0
                continue
            ps = parents[v - L]
            window, local_old, leaf, remote = [], [], [], []
            for t_i, p in enumerate(ps):
                if p < L:
                    leaf.append(t_i)
                elif round_of[p] > c_r:
                    assert core_of[p] == q and round_of[p] < r
                    window.append(t_i)
                elif core_of[p] == q:
                    local_old.append(t_i)
                else:
                    remote.append(t_i)
            own_side = window + local_old + leaf  # priority for own positions
            need = amax
            own_taps = own_side[:need]
            assert len(own_taps) == need, (
                f"slot {s} core {q}: cannot fill {need} own positions"
            )
            rest = [t_i for t_i in range(P) if t_i not in own_taps]
            # rest must be hbuf-eligible: leaf or covered computed
            for t_i in rest:
                p = ps[t_i]
                assert p < L or round_of[p] <= c_r, "window tap in hbuf position"
            order = own_taps + rest
            perm[q, s] = order
            for pos, t_i in enumerate(order):
                p = ps[t_i]
                if pos < amax:  # own region
                    if p < L:
                        gidx_vals[q, s, pos] = p * 128
                    else:
                        gidx_vals[q, s, pos] = (L + slot_of[p]) * 128
                else:  # hbuf region
                    if p < L:
                        gidx_vals[q, s, pos] = p * 128
                    else:
                        gidx_vals[q, s, pos] = hbuf_row(core_of[p], slot_of[p])
    assert a_list.max() <= A_MAX

    # ---- weight relayout: [M, o, c, p] -> [128, 16(ktile=2*pos+h), 2(oh), 128]
    # with tap permutation applied per (core, slot).
    W4 = W.transpose(0, 3, 2, 1).reshape(M, 8, 2, 128, 2, 128)
    # W4[m, tap, h(c//128), c%128, oh, o%128]
    if W_DTYPE == "f8e3":
        W4q = np.clip(W4 * W_SCALE, -15.5, 15.5).astype(ml_dtypes.float8_e3m4)
        wdt = ml_dtypes.float8_e3m4
    else:
        W4q = (W4 * W_SCALE).astype(np.float16)
        wdt = np.float16
    xt_host = np.ascontiguousarray(
        (x.transpose(2, 1, 0) / W_SCALE)
        .reshape(L, 2, 128, 32)
        .transpose(0, 2, 1, 3)
        .reshape(L * 128, 64)
        .astype(np.float16)
    )

    in_maps = []
    for q in range(NCORES):
        nodes_q = node_of_coreslot[q]
        Wq = np.zeros((S, 128, 16, 2, 128), wdt)
        bq = np.zeros((S, 2, 128), np.float32)
        for s in range(S):
            v = nodes_q[s]
            if v < 0:
                continue
            # build [128, 16, 2, 128]: ktile = 2*pos + h -> W4q[v, perm[pos], h]
            wv = W4q[v - L]  # [8, 2, 128, 2, 128]
            wp_ = wv[perm[q, s]]  # [8(pos), 2(h), 128(part), 2(oh), 128(o)]
            Wq[s] = wp_.reshape(16, 128, 2, 128).transpose(1, 0, 2, 3)
            bq[s] = (b[v - L] / W_SCALE).reshape(2, 128)
        bq2 = np.ascontiguousarray(bq.transpose(2, 0, 1).reshape(128, 2 * S))
        gq = np.ascontiguousarray(
            gidx_vals[q].reshape(1, 8 * S).astype(np.int32)
        )
        in_maps.append({"wbuf": Wq, "xt": xt_host, "bbuf": bq2, "gidx": gq})

    key = (
        L,
        tuple(k_list),
        tuple(a_list.tolist()),
        tuple(tops),
        W_DTYPE,
    )
    if key not in _BUILD_CACHE:
        import time as _time

        _t0 = _time.time()
        _BUILD_CACHE[key] = _build_bass(
            L, k_list, S, a_list.tolist(), top_of_round, wins
        )
        print(f"[kernel] bass build took {_time.time() - _t0:.1f}s", flush=True)
    nc = _BUILD_CACHE[key]

    global LAST_RUN
    LAST_RUN = (nc, in_maps)

    results = run_bass_kernel_spmd(nc, in_maps, core_ids=list(range(NCORES))).results

    out = np.zeros((N, C, L + M), np.float32)
    out[:, :, :L] = x
    for q in range(NCORES):
        yq = (
            np.asarray(results[q]["yout"])
            .astype(np.float32)
            .reshape(S, 128, 2, 32)
            .transpose(0, 3, 2, 1)
            .reshape(S, 32, 256)
        ) * W_SCALE
        for s in range(S):
            v = node_of_coreslot[q, s]
            if v >= 0:
                out[:, :, v] = yq[s]  # v is already L-based
    return out


# revision 18
# speedup vs baseline: 1.0610x; 1.0526x over previous
"""Trainium2 Bass kernel for nn_DAGLayer (gnn_message_passing).

Problem: out buffer holds L leaf columns followed by M computed nodes.
Node i gathers P=8 parent columns (each [N, C]) from the buffer, applies a
per-node dense map y = einsum('ncp,ocp->no', g, W[i]) + b[i], and appends y.

Strategy (8 NeuronCores, one SPMD program):
  - The WHOLE history (leaves + AllGathered windows + this core's outputs)
    lives in ONE resident SBUF tile `hist` [128 part (c%128), cols, 64
    (ch,n)].  A parent "gather" is just a register column-offset on the
    matmul's moving operand - zero per-tap DMA instructions (the previous
    design spent ~0.7us of DGE/sequencer time per tap and was issue-bound).
  - Host schedules nodes into rounds with a LOCALITY rule: a node may depend
    on rounds newer than the last readable AllGather only via parents
    computed on the SAME core (their y is already in hist).  AllGathers
    cover fixed-size windows of WBAR slots; after an AG completes, 8
    per-rank bulk DMAs copy the window into everyone's hist (emitted one
    round late so earlier matmuls don't conservatively wait on them).
  - Weights are fp8 e3m4 scaled by 512 (uniform W fits 4 mantissa bits;
    measured DAG rel err ~8.5e-3).  All stored activations are at 1/512
    scale so the scaling cancels; the host rescales the output by 512.
  - Per slot: one 8-register index load on the PE sequencer, 32 accumulating
    matmuls (stationary fp8 weights, moving fp16 hist slices), two
    psum->hist activations (bias + 1/512), one y staging DMA for the AG.
  - Engines: weights + y staging + AGs on gpsimd; leaf load + window copies
    on sync; activations scalar; matmuls + index loads tensor.

The kernel is self-contained; the schedule is derived from the inputs at
run time on the host.
"""

import os

import numpy as np

os.environ.setdefault("NEURON_COMPILE_CACHE_URL", "/root/neuron_cache")

NCORES = 8
AG_T = 6          # AllGather threshold: place an AG top once >= AG_T slots pending
COV_LAG = 1       # AG at top t is readable from round t+COV_LAG
W_PF = 6          # weight DMAs emitted this many rounds ahead of use
W_SCALE = 512.0   # weight scale for e3m4; activations stored at 1/W_SCALE
W_DTYPE = "f8e3"  # "f8e3" or "f16"

_BUILD_CACHE = {}


# ---------------------------------------------------------------- scheduler
def _compute_schedule(parents, L, M):
    """Assign nodes to (round, core) with the locality rule; pick AG tops."""
    NC = NCORES
    children = [[] for _ in range(L + M)]
    for i in range(M):
        for p in set(parents[i].tolist()):
            children[p].append(L + i)
    height = np.zeros(L + M, np.int64)
    for v in range(L + M - 1, L - 1, -1):
        for ch in children[v]:
            height[v] = max(height[v], height[ch] + 1)
    round_of = np.full(L + M, -1, np.int64)
    core_of = np.full(L + M, -1, np.int64)
    round_of[:L] = -(10**9)
    unsched = set(range(L, L + M))
    rounds = []
    tops = []
    slots_done = 0
    covered_slots = 0
    r = 0
    guard = 0
    while unsched and guard < 400:
        guard += 1
        assert len(rounds) == r
        if rounds and (slots_done - covered_slots) >= AG_T:
            tops.append(r)
            covered_slots = slots_done
        c_r = -1
        for t in tops:
            if t <= r - COV_LAG:
                c_r = t - 1
        free, forced = [], {c: [] for c in range(NC)}
        for v in unsched:
            ps = set(parents[v - L].tolist())
            ok = True
            fcores = set()
            for p in ps:
                pr = round_of[p]
                if p < L:
                    continue
                if pr < 0:
                    ok = False
                    break
                if pr > c_r:
                    if pr <= r - 1:
                        fcores.add(core_of[p])
                    else:
                        ok = False
                        break
            if not ok:
                continue
            if len(fcores) > 1:
                continue
            if len(fcores) == 1:
                forced[list(fcores)[0]].append(v)
            else:
                free.append(v)
        total = len(free) + sum(len(x) for x in forced.values())
        if total == 0:
            rounds.append(np.full((NC, 0), -1, np.int64))
            r += 1
            if r > 300:
                break
            continue
        maxk = 0
        for k in range(1, 65):
            if sum(max(0, k - len(forced[c])) for c in range(NC)) <= len(free):
                maxk = k
        k = min(maxk if maxk else 1, (total + NC - 1) // NC)
        free.sort(key=lambda v: -height[v])
        for c in forced:
            forced[c].sort(key=lambda v: -height[v])
        node_of = np.full((NC, k), -1, np.int64)
        fi = 0
        for c in range(NC):
            take = list(forced[c][:k])
            while len(take) < k and fi < len(free):
                take.append(free[fi])
                fi += 1
            for m, v in enumerate(take):
                node_of[c, m] = v
                round_of[v] = r
                core_of[v] = c
                unsched.discard(v)
        rounds.append(node_of)
        slots_done += k
        r += 1
    assert not unsched, "scheduler failed to place all nodes"
    return rounds, tops, round_of, core_of


# ---------------------------------------------------------------- bass build
def _build_bass(
    L, k_list, S, NW, WBAR, top_info, copy_info, NH, OWNBASE, slot_window
):
    """top_info[r] = (j, o0, o1) if AG for window j launches at the top of
    round r; copy_info[r] = j whose hist copy is emitted at round r;
    slot_window[s] = (j, pos) or (None, None)."""
    import concourse.bacc as bacc
    import concourse.bass as bass
    import concourse.mybir as mybir
    import concourse.tile as tile

    f16 = mybir.dt.float16
    f32 = mybir.dt.float32
    i32 = mybir.dt.int32
    f8 = mybir.dt.float8e3 if W_DTYPE == "f8e3" else mybir.dt.float16

    nc = bacc.Bacc(num_devices=NCORES, num_swdge_queues=4)

    wbuf = nc.dram_tensor("wbuf", [S, 128, 16, 2, 128], f8, kind="ExternalInput")
    xt = nc.dram_tensor("xt", [128, L, 64], f16, kind="ExternalInput")
    bbuf = nc.dram_tensor("bbuf", [128, 2 * S], f32, kind="ExternalInput")
    gidx = nc.dram_tensor("gidx", [1, 8 * S], i32, kind="ExternalInput")
    yout = nc.dram_tensor("yout", [128, S, 64], f16, kind="ExternalOutput")
    # AG staging: per window, this rank's stream is [128 (p), WBAR, 64]
    own3 = nc.dram_tensor("own3", [NW, 128, WBAR, 64], f16)
    hbuf = nc.dram_tensor(
        "hbuf", [NW, 8, 128, WBAR, 64], f16, addr_space="Shared"
    )
    rg = [list(range(NCORES))]

    w_bufs = max(sum(k_list[r : r + W_PF + 1]) for r in range(len(k_list))) + 1

    with tile.TileContext(nc) as tc:
        with (
            tc.tile_pool(name="const", bufs=1) as constp,
            tc.tile_pool(name="w", bufs=w_bufs) as wp,
            tc.tile_pool(name="py", bufs=4, space="PSUM") as pyp,
        ):
            b_sb = constp.tile([128, 2 * S], f32)
            nc.sync.dma_start(b_sb[:], bbuf[:])
            gidx_sb = constp.tile([1, 8 * S], i32)
            nc.sync.dma_start(gidx_sb[:], gidx[:])
            hist = constp.tile([128, NH, 64], f16)
            nc.sync.dma_start(hist[:, 0:L, :], xt[:])

            R = len(k_list)
            off_of = [0]
            for k_r in k_list:
                off_of.append(off_of[-1] + k_r)
            w_tiles = {}

            def emit_weights(r):
                if r >= R:
                    return
                for s in range(off_of[r], off_of[r + 1]):
                    w_t = wp.tile([128, 16, 2, 128], f8, tag="w")
                    nc.gpsimd.dma_start(w_t[:], wbuf[s])
                    w_tiles[s] = w_t

            for r0 in range(min(W_PF, R)):
                emit_weights(r0)

            for r, k_r in enumerate(k_list):
                emit_weights(r + W_PF)
                # window copy for an AG launched >= 1 round ago (true dep now)
                if copy_info[r] is not None:
                    j = copy_info[r]
                    for q in range(NCORES):
                        base = L + (j * 8 + q) * WBAR
                        nc.sync.dma_start(
                            hist[:, base : base + WBAR, :], hbuf[j, q]
                        )
                # AllGather launch at this round's top
                if top_info[r] is not None:
                    j, o0, o1 = top_info[r]
                    nc.gpsimd.collective_compute(
                        "AllGather",
                        mybir.AluOpType.bypass,
                        replica_groups=rg,
                        ins=[own3[j]],
                        outs=[hbuf[j]],
                    )
                # compute
                for s in range(off_of[r], off_of[r + 1]):
                    w_t = w_tiles.pop(s)
                    cols = gidx_sb[0:1, 8 * s : 8 * s + 8]
                    _, vals = nc.values_load_multi_w_load_instructions(
                        cols,
                        engines=[nc.tensor.engine],
                        min_val=0,
                        max_val=NH - 1,
                        skip_runtime_bounds_check=True,
                    )
                    pys = [
                        pyp.tile([128, 32], f32, tag="py", name=f"py{oh}")
                        for oh in range(2)
                    ]
                    for i in range(16):
                        t, h = i // 2, i % 2
                        rhs = hist[
                            :, bass.ds(vals[t], 1), h * 32 : (h + 1) * 32
                        ]
                        for oh in range(2):
                            nc.tensor.matmul(
                                pys[oh][:],
                                w_t[:, 2 * t + h, oh, :],
                                rhs,
                                start=(i == 0),
                                stop=(i == 15),
                            )
                    for oh in range(2):
                        bias = b_sb[:, 2 * s + oh : 2 * s + oh + 1]
                        nc.scalar.activation(
                            hist[:, OWNBASE + s, oh * 32 : (oh + 1) * 32],
                            pys[oh][:],
                            mybir.ActivationFunctionType.Identity,
                            bias=bias,
                            scale=float(1.0 / W_SCALE),
                        )
                    # stage y for its AG window (j, slot position)
                    j, pos = slot_window[s]
                    if j is not None:
                        nc.gpsimd.dma_start(
                            own3[j, :, pos, :], hist[:, OWNBASE + s, :]
                        )

            nc.gpsimd.dma_start(yout[:], hist[:, OWNBASE : OWNBASE + S, :])
    nc.compile()
    return nc


# ---------------------------------------------------------------- host glue
def kernel(x, W, b, parents):
    import ml_dtypes
    from concourse.bass_utils import run_bass_kernel_spmd

    x = np.ascontiguousarray(np.asarray(x), dtype=np.float32)
    W = np.ascontiguousarray(np.asarray(W), dtype=np.float32)
    b = np.ascontiguousarray(np.asarray(b), dtype=np.float32)
    parents = np.asarray(parents).astype(np.int64)

    N, C, L = x.shape
    M, O, C2, P = W.shape
    assert (N, C, O, C2, P) == (32, 256, 256, 256, 8), "kernel hardcodes these dims"

    rounds, tops, round_of, core_of = _compute_schedule(parents, L, M)
    k_list = [nd.shape[1] for nd in rounds]
    R = len(rounds)
    S = sum(k_list)
    off_of_round = np.concatenate([[0], np.cumsum(k_list)]).astype(np.int64)

    slot_of = np.full(L + M, -1, np.int64)
    node_of_coreslot = np.full((NCORES, S), -1, np.int64)
    round_of_slot = np.zeros(S, np.int64)
    for r, nd in enumerate(rounds):
        for m in range(nd.shape[1]):
            s = off_of_round[r] + m
            round_of_slot[s] = r
            for q in range(NCORES):
                v = nd[q, m]
                if v >= 0:
                    slot_of[v] = s
                    node_of_coreslot[q, s] = v

    # AG windows (constant padded size WBAR).  top at round t covers
    # per-core slots [o0, o1); its hist copy is emitted at round t+1.
    wins = []
    top_info = [None] * R
    copy_info = [None] * R
    prev = 0
    for t in tops:
        o0, o1 = prev, int(off_of_round[t])
        if o1 > o0:
            j = len(wins)
            wins.append((o0, o1))
            top_info[t] = (j, o0, o1)
            if t + 1 < R:
                copy_info[t + 1] = j
            prev = o1
    NW = max(1, len(wins))
    WBAR = max((o1 - o0 for o0, o1 in wins), default=1)
    win_of_slot = {}
    for j, (o0, o1) in enumerate(wins):
        for s in range(o0, o1):
            win_of_slot[s] = (j, s - o0)
    slot_window = {s: win_of_slot.get(s, (None, None)) for s in range(S)}

    NH = L + 8 * WBAR * NW + S
    OWNBASE = L + 8 * WBAR * NW

    def cov(r):
        c = -1
        for t in tops:
            if t <= r - COV_LAG:
                c = t - 1
        return c

    # hist column index of each tap (per core): leaves [0,L); local parents
    # via the own section; covered remote parents via their AG window copy.
    gidx_vals = np.zeros((NCORES, S, P), np.int64)
    for s in range(S):
        r = round_of_slot[s]
        c_r = cov(r)
        for q in range(NCORES):
            v = node_of_coreslot[q, s]
            if v < 0:
                continue
            for t_i, p in enumerate(parents[v - L]):
                if p < L:
                    gidx_vals[q, s, t_i] = p
                elif core_of[p] == q:
                    gidx_vals[q, s, t_i] = OWNBASE + slot_of[p]
                else:
                    pr = round_of[p]
                    assert pr <= c_r, f"remote uncovered parent {p} @ slot {s}"
                    ps = slot_of[p]
                    j, pos = win_of_slot[ps]
                    gidx_vals[q, s, t_i] = L + (j * 8 + core_of[p]) * WBAR + pos

    # weight relayout [M, o, c, p] -> [128, 16 (ktile=2t+h), 2 (oh), 128]
    W4 = W.transpose(0, 3, 2, 1).reshape(M, 8, 2, 128, 2, 128)
    if W_DTYPE == "f8e3":
        W4q = np.clip(W4 * W_SCALE, -15.5, 15.5).astype(ml_dtypes.float8_e3m4)
        wdt = ml_dtypes.float8_e3m4
    else:
        W4q = (W4 * W_SCALE).astype(np.float16)
        wdt = np.float16
    # xt: [128 (c%128), L, 64 (ch, n)]
    xt_host = np.ascontiguousarray(
        (x.transpose(1, 2, 0) / W_SCALE)          # [C, L, N]
        .reshape(2, 128, L, 32)                    # [ch, c%128, L, n]
        .transpose(1, 2, 0, 3)                     # [c%128, L, ch, n]
        .reshape(128, L, 64)
        .astype(np.float16)
    )

    in_maps = []
    for q in range(NCORES):
        nodes_q = node_of_coreslot[q]
        Wq = np.zeros((S, 128, 16, 2, 128), wdt)
        bq = np.zeros((S, 2, 128), np.float32)
        for s in range(S):
            v = nodes_q[s]
            if v < 0:
                continue
            wv = W4q[v - L]  # [8 (t), 2 (h), 128 (part), 2 (oh), 128 (o)]
            Wq[s] = wv.reshape(16, 128, 2, 128).transpose(1, 0, 2, 3)
            bq[s] = (b[v - L] / W_SCALE).reshape(2, 128)
        bq2 = np.ascontiguousarray(bq.transpose(2, 0, 1).reshape(128, 2 * S))
        gq = np.ascontiguousarray(
            gidx_vals[q].reshape(1, 8 * S).astype(np.int32)
        )
        in_maps.append({"wbuf": Wq, "xt": xt_host, "bbuf": bq2, "gidx": gq})

    key = (L, tuple(k_list), tuple(tops), NW, WBAR, W_DTYPE)
    if key not in _BUILD_CACHE:
        import time as _time

        _t0 = _time.time()
        _BUILD_CACHE[key] = _build_bass(
            L, k_list, S, NW, WBAR, top_info, copy_info, NH, OWNBASE,
            slot_window,
        )
        print(f"[kernel] bass build took {_time.time() - _t0:.1f}s", flush=True)
    nc = _BUILD_CACHE[key]

    global LAST_RUN
    LAST_RUN = (nc, in_maps)

    results = run_bass_kernel_spmd(nc, in_maps, core_ids=list(range(NCORES))).results

    out = np.zeros((N, C, L + M), np.float32)
    out[:, :, :L] = x
    for q in range(NCORES):
        # yout [128 (c%128), S, 64 (ch, n)]
        yq = (
            np.asarray(results[q]["yout"])
            .astype(np.float32)
            .reshape(128, S, 2, 32)
            .transpose(1, 3, 2, 0)                # [s, n, ch, c%128]
            .reshape(S, 32, 256)
        ) * W_SCALE
        for s in range(S):
            v = node_of_coreslot[q, s]
            if v >= 0:
                out[:, :, v] = yq[s]
    return out
